# revision 18
# baseline (speedup 1.0000x reference)
"""9x9 morphological dilation (sliding-window max, SAME padding) on Trainium2.

Input : label (16, 1024, 1024, 1) float32, values in [0, 1).
Output: same shape; out[b,i,j] = max over the 9x9 window centered at (i,j),
        clipped to the image (cv2-style border handling for dilate).

Strategy (per NeuronCore; batch is data-parallel over 8 cores, 2 images/core):
  - All device compute and HBM traffic is bf16.  max() commutes with
    round-to-nearest, so out == rn_bf16(exact out): rel err <= 2^-9 ~ 0.2%,
    far inside the 2e-2 gate.  The host converts f32->bf16 on the way in and
    bf16->f32 on the way out; device DMA traffic is halved and bf16
    tensor_tensor runs in the DVE's packed perf mode, which fp32 cannot use.
  - Layout: partition p holds img = p%2, row-band q = p//2 (image rows
    16q..16q+15); free dim is r-major with u = cw+8 padded columns per chunk
    (+-4 halo, zero pads at image edges; zero is a valid -inf substitute
    since inputs >= 0).  Interleaving the two images on even/odd partitions
    makes "next row-band" = "partition p+2" for BOTH images, so every
    cross-partition carry is ONE partition-shifted DMA (and the image-bottom
    partitions 126/127 keep permanently-zero carry rows).
  - Each tree tile carries extra rows at the end (X:+1, T2:+2, T4:+4,
    T8:+1) holding the next band's leading rows.
  - Vertical 9-max: log tree (row shifts +1,+2,+4,+1 on X,T2,T4,T8).  Every
    level is split into a carry-independent BULK op and a small BOUNDARY op;
    each carry DMA is issued right after the bulk rows it reads, and the
    boundary ops run several ops later, so the ~3us carry-DMA flight time
    hides under other DVE work instead of stalling the pipe at every level.
  - Horizontal 9-max: log tree with column shifts +1,+2,+4,+8.  A +1 bf16
    shift is 2B-misaligned (drops the op out of the packed mode), so the odd
    shift is materialized once per chunk as a shifted copy on the (otherwise
    idle) Activation engine: Y[m] = R9[m+1].  The rest (H2=max(R9,Y),
    H4=max(H2,H2>>2), H8=max(H4,H4>>4), OUT=max(H8,R9>>8)) uses even shifts
    only.  The H ops of chunk i-1 are emitted BETWEEN the bulk and boundary
    tree ops of chunk i as fillers for the carry latency.
  - HWDGE descriptor generation is a single serialized device costing
    ~625ns per dma_start regardless of size, so DMAs are maximally merged:
    per chunk 2 loads, 4 carries, 1 top-row stash, 4 stores (2 of them on
    the Pool/SWDGE path, which bypasses HWDGE).
  - R9[r] covers rows R..R+8 -> output row R+4; recentering happens in the
    store offsets.  Output rows 0..3 (clipped top windows) are built from
    tree intermediates and stashed into the otherwise-unused rows
    (p in {126,127}, r>=12) so they ride the same horizontal pass/stores.
"""

import numpy as np

B, H, W = 16, 1024, 1024
NCORES = 8
IMGS = 2            # images per core
RB = 16             # rows per partition
CHUNKS = [344, 344, 336]
assert sum(CHUNKS) == W
NCH = len(CHUNKS)
UM = max(CHUNKS) + 8

_CACHE = {}


def _build(reps=1):
    import concourse.bacc as bacc
    import concourse.tile as tile
    import concourse.mybir as mybir

    bf16 = mybir.dt.bfloat16

    nc = bacc.Bacc("TRN2", target_bir_lowering=False, debug=False, num_devices=1)
    x = nc.dram_tensor("x", [IMGS, H, W], bf16, kind="ExternalInput").ap()
    y = nc.dram_tensor("y", [IMGS, H, W], bf16, kind="ExternalOutput").ap()

    xv = [x[i].rearrange("(q r) c -> q r c", r=RB) for i in range(IMGS)]
    chunk_off = np.cumsum([0] + CHUNKS[:-1]).tolist()

    with tile.TileContext(nc) as tc:
        with tc.tile_pool(name="pt", bufs=1) as pt:

            def t3(rows, tag):
                t = pt.tile([128, rows * UM], bf16, tag=tag)
                return t.rearrange("p (r u) -> p r u", u=UM)

            XT = [t3(17, "x0"), t3(17, "x1")]
            T2T = t3(18, "t2")
            T4T = t3(20, "t4")
            T8T = t3(17, "t8")
            R9T = [t3(16, "r90"), t3(16, "r91")]
            YT = [t3(16, "y0"), t3(16, "y1")]
            H2T = t3(16, "h2")
            H4T = t3(16, "h4")
            H8T = t3(16, "h8")
            OT = [t3(16, "o0"), t3(16, "o1")]
            TPT = [t3(4, "tp0"), t3(4, "tp1")]

            # carry rows start zero; partitions 0:126 are rewritten by the
            # per-chunk carry DMAs, the image-bottom partitions (126,127)
            # keep the zeros forever (partition-sliced memsets are not legal
            # BIR, so clear all 128 partitions)
            for t, r0, r1 in ((XT[0], 16, 17), (XT[1], 16, 17), (T2T, 16, 18),
                              (T4T, 16, 20), (T8T, 16, 17)):
                nc.gpsimd.memset(t[:, r0:r1, :], 0.0)

            def uof(it):
                return CHUNKS[it % NCH] + 8

            def load(it):
                ch = it % NCH
                cw = CHUNKS[ch]
                u = cw + 8
                c0 = chunk_off[ch]
                clo = max(0, c0 - 4)
                chi = min(W, c0 + cw + 4)
                ncols = chi - clo
                ulo = clo - (c0 - 4)
                x3 = XT[it % 2]
                if ulo > 0:
                    nc.gpsimd.memset(x3[:, 0:RB, 0:ulo], 0.0)
                if ulo + ncols < u:
                    nc.gpsimd.memset(x3[:, 0:RB, ulo + ncols:u], 0.0)
                row_groups = [(0, 4), (4, 8), (8, 12), (12, RB)] if it == 0 else [(0, RB)]
                for rlo, rhi in row_groups:
                    for img in range(IMGS):
                        nc.scalar.dma_start(
                            out=x3[img:img + 127:2, rlo:rhi, ulo:ulo + ncols],
                            in_=xv[img][:, rlo:rhi, clo:chi],
                        )
                return x3

            def xcarry(it):
                # X carry: row 16 <- next band's row 0 (both images at once);
                # issued a full chunk before the boundary op that reads it
                u = uof(it)
                x3 = XT[it % 2]
                nc.sync.dma_start(out=x3[0:126, 16:17, 0:u], in_=x3[2:128, 0:1, 0:u])

            def tree_bulk(it):
                # carry-independent rows of every level; each carry DMA is
                # issued as soon as the rows it reads exist
                u = uof(it)
                x3 = XT[it % 2]
                sync = nc.sync
                if it == 0:
                    # first chunk: quarter T2 so compute starts mid-load
                    nc.vector.tensor_max(T2T[:, 0:3, 0:u], x3[:, 0:3, 0:u], x3[:, 1:4, 0:u])
                    nc.vector.tensor_max(T2T[:, 3:7, 0:u], x3[:, 3:7, 0:u], x3[:, 4:8, 0:u])
                    nc.vector.tensor_max(T2T[:, 7:11, 0:u], x3[:, 7:11, 0:u], x3[:, 8:12, 0:u])
                    nc.vector.tensor_max(T2T[:, 11:15, 0:u], x3[:, 11:15, 0:u], x3[:, 12:16, 0:u])
                else:
                    nc.vector.tensor_max(T2T[:, 0:15, 0:u], x3[:, 0:15, 0:u], x3[:, 1:16, 0:u])
                sync.dma_start(out=T2T[0:126, 16:18, 0:u], in_=T2T[2:128, 0:2, 0:u])
                nc.vector.tensor_max(T4T[:, 0:13, 0:u], T2T[:, 0:13, 0:u], T2T[:, 2:15, 0:u])
                sync.dma_start(out=T4T[0:126, 16:20, 0:u], in_=T4T[2:128, 0:4, 0:u])
                nc.vector.tensor_max(T8T[:, 0:9, 0:u], T4T[:, 0:9, 0:u], T4T[:, 4:13, 0:u])
                sync.dma_start(out=T8T[0:126, 16:17, 0:u], in_=T8T[2:128, 0:1, 0:u])
                r9 = R9T[it % 2]
                nc.vector.tensor_max(r9[:, 0:8, 0:u], T8T[:, 0:8, 0:u], T8T[:, 1:9, 0:u])
                # top output rows 0..2 for both images (partitions 0,1 = q0)
                tp = TPT[it % 2]
                nc.vector.tensor_max(tp[0:2, 0:1, 0:u], T4T[0:2, 0:1, 0:u], x3[0:2, 4:5, 0:u])
                nc.vector.tensor_max(tp[0:2, 1:2, 0:u], T4T[0:2, 0:1, 0:u], T2T[0:2, 4:5, 0:u])
                nc.vector.tensor_max(tp[0:2, 2:3, 0:u], T4T[0:2, 0:1, 0:u], T4T[0:2, 3:4, 0:u])

            def tree_bnd(it):
                # boundary rows: consume the carry rows issued in tree_bulk
                u = uof(it)
                x3 = XT[it % 2]
                nc.vector.tensor_max(T2T[:, 15:16, 0:u], x3[:, 15:16, 0:u], x3[:, 16:17, 0:u])
                nc.vector.tensor_max(T4T[:, 13:16, 0:u], T2T[:, 13:16, 0:u], T2T[:, 15:18, 0:u])
                nc.vector.tensor_max(T8T[:, 9:16, 0:u], T4T[:, 9:16, 0:u], T4T[:, 13:20, 0:u])
                r9 = R9T[it % 2]
                nc.vector.tensor_max(r9[:, 8:16, 0:u], T8T[:, 8:16, 0:u], T8T[:, 9:17, 0:u])

            def tree_tail(it):
                # ACT copies + stash after the boundary ops
                s = it % 2
                cw = CHUNKS[it % NCH]
                u = uof(it)
                r9 = R9T[s]
                tp = TPT[s]
                nc.scalar.copy(tp[0:2, 3:4, 0:u], T8T[0:2, 0:1, 0:u])
                # odd horizontal shift on ACT: Y[m] = R9[m+1]; rows 0:12 don't
                # overlap the stash, so only rows 12:16 wait on it
                nc.scalar.copy(YT[s][:, 0:12, 0:cw + 6], r9[:, 0:12, 1:cw + 7])
                nc.sync.dma_start(out=r9[126:128, 12:16, 0:u], in_=tp[0:2, 0:4, 0:u])
                nc.scalar.copy(YT[s][:, 12:16, 0:cw + 6], r9[:, 12:16, 1:cw + 7])

            def hstage(it):
                s = it % 2
                ch = it % NCH
                cw = CHUNKS[ch]
                c0 = chunk_off[ch]
                r9 = R9T[s]
                o3 = OT[s]
                ymains = [
                    y[img][4:4 + 63 * RB, c0:c0 + cw].rearrange("(q r) c -> q r c", r=RB)
                    for img in range(IMGS)
                ]

                def store_main(rlo, rhi, split=False):
                    for img in range(IMGS):
                        # on the drain path, route one store via SWDGE (Pool)
                        # so the two final stores don't serialize on HWDGE
                        eng = nc.gpsimd if (split and img == 1) else nc.sync
                        eng.dma_start(
                            out=ymains[img][:, rlo:rhi, :],
                            in_=o3[img:img + 125:2, rlo:rhi, 0:cw],
                        )

                def store_tail():
                    # bottom rows 1012..1023 at (p 126/127, r 0..11); SWDGE
                    # (Pool) path keeps these small stores off the HWDGE queue
                    nc.gpsimd.dma_start(
                        out=y[:, 1012:1024, c0:c0 + cw], in_=o3[126:128, 0:12, 0:cw]
                    )

                def store_top():
                    # top rows 0..3 from the stash (p 126/127, r 12..15)
                    nc.gpsimd.dma_start(
                        out=y[:, 0:4, c0:c0 + cw], in_=o3[126:128, 12:16, 0:cw]
                    )

                def h2(hlo, hhi):
                    nc.vector.tensor_max(
                        H2T[:, hlo:hhi, 0:cw + 6],
                        r9[:, hlo:hhi, 0:cw + 6],
                        YT[s][:, hlo:hhi, 0:cw + 6],
                    )

                def h4(hlo, hhi):
                    nc.vector.tensor_max(
                        H4T[:, hlo:hhi, 0:cw + 4],
                        H2T[:, hlo:hhi, 0:cw + 4],
                        H2T[:, hlo:hhi, 2:cw + 6],
                    )

                def h8(hlo, hhi):
                    nc.vector.tensor_max(
                        H8T[:, hlo:hhi, 0:cw],
                        H4T[:, hlo:hhi, 0:cw],
                        H4T[:, hlo:hhi, 4:cw + 4],
                    )

                def merge(hlo, hhi):
                    nc.vector.tensor_max(
                        o3[:, hlo:hhi, 0:cw],
                        H8T[:, hlo:hhi, 0:cw],
                        r9[:, hlo:hhi, 8:cw + 8],
                    )

                return (h2, h4, h8, merge, store_main, store_tail, store_top)

            def emit_h_part1(it):
                h2, h4, h8, merge, sm, stl, stp = hstage(it)
                h2(0, 16)
                h4(0, 16)

            def emit_h_part2(it):
                h2, h4, h8, merge, sm, stl, stp = hstage(it)
                h8(0, 16)
                merge(0, 16)
                sm(0, 16)
                stl()
                stp()

            def emit_h_last(it):
                # final chunk: drain in halves/quarters so stores overlap the
                # remaining merges instead of queueing after them
                h2, h4, h8, merge, sm, stl, stp = hstage(it)
                h2(0, 8)
                h4(0, 8)
                h8(0, 8)
                merge(0, 8)
                sm(0, 8)
                h2(8, 16)
                h4(8, 16)
                h8(8, 16)
                merge(8, 12)
                sm(8, 12)
                stl()
                merge(12, 16)
                sm(12, 16, split=True)
                stp()

            niter = NCH * reps
            load(0)
            xcarry(0)
            for it in range(niter):
                if it + 1 < niter:
                    load(it + 1)
                    xcarry(it + 1)
                tree_bulk(it)
                if it > 0:
                    emit_h_part1(it - 1)
                tree_bnd(it)
                if it > 0:
                    emit_h_part2(it - 1)
                tree_tail(it)
            emit_h_last(niter - 1)

    nc.compile()
    return nc


def kernel(label):
    import ml_dtypes

    lab = np.ascontiguousarray(
        np.asarray(label, dtype=np.float32).reshape(B, H, W)
    ).astype(ml_dtypes.bfloat16)
    if "nc" not in _CACHE:
        _CACHE["nc"] = _build()
    nc = _CACHE["nc"]

    from concourse.bass_utils import run_bass_kernel_spmd

    in_maps = [{"x": lab[IMGS * c:IMGS * (c + 1)]} for c in range(NCORES)]
    res = run_bass_kernel_spmd(nc, in_maps, core_ids=list(range(NCORES)))
    out = np.concatenate(
        [np.asarray(res.results[c]["y"]).astype(np.float32) for c in range(NCORES)],
        axis=0,
    )
    return out.reshape(B, H, W, 1)


# revision 19
# speedup vs baseline: 1.0340x; 1.0340x over previous
"""9x9 morphological dilation (sliding-window max, SAME padding) on Trainium2.

Input : label (16, 1024, 1024, 1) float32, values in [0, 1).
Output: same shape; out[b,i,j] = max over the 9x9 window centered at (i,j),
        clipped to the image (cv2-style border handling for dilate).

Strategy (per NeuronCore; batch is data-parallel over 8 cores, 2 images/core):
  - All device compute and HBM traffic is bf16.  max() commutes with
    round-to-nearest, so out == rn_bf16(exact out): rel err <= 2^-9 ~ 0.2%,
    far inside the 2e-2 gate.  The host converts f32->bf16 on the way in and
    bf16->f32 on the way out; device DMA traffic is halved and every DVE
    tensor_tensor op runs in the 2x_1P perf mode (2 elem/cycle/lane), which
    fp32 tensor_tensor cannot use.
  - Layout: partition p holds img = p%2, row-band q = p//2 (image rows
    16q..16q+15); free dim is r-major with u = cw+12 padded columns per
    chunk (+-4 halo, zero pads at image edges; zero is a valid -inf
    substitute since inputs >= 0).  Interleaving the two images on even/odd
    partitions makes "next row-band" = "partition p+2" for BOTH images, so
    every cross-partition carry is ONE partition-shifted DMA (and the
    image-bottom partitions 126/127 keep permanently-zero carry rows).
  - Each tree tile carries extra rows at the end (X:+1, T2:+2, T4:+4,
    T8:+1) that hold the next band's leading rows, so every tree level is a
    single full-tile tensor_max (no separate boundary op).
  - Vertical 9-max: log tree (row shifts +1,+2,+4,+1 applied to X,T2,T4,T8).
    Row shifts keep the column offset 0, so all operands stay 4B-aligned ->
    2x mode.
  - Horizontal 9-max: log tree with column shifts +1,+2,+4,+8.  A +1 bf16
    shift is 2B-misaligned and would drop the whole op to 1x, so the odd
    shift is materialized once per chunk as a shifted copy on the (otherwise
    idle) Activation engine: Y[m] = R9[m+1].  The rest (H2=max(R9,Y),
    H4=max(H2,H2>>2), H8=max(H4,H4>>4), OUT=max(H8,R9>>8)) is all even ->
    2x on the DVE.
  - HWDGE descriptor generation is a single serialized device costing
    ~625ns per dma_start regardless of size, so DMAs are maximally merged:
    per chunk 2 loads, 4 carries, 1 top-row stash, 4 stores (2 of them on
    the Pool/SWDGE path, which bypasses HWDGE).
  - R9[r] covers rows R..R+8 -> output row R+4; recentering happens in the
    store offsets.  Output rows 0..3 (clipped top windows) are built from
    tree intermediates and stashed into the otherwise-unused rows
    (p in {126,127}, r>=12) so they ride the same horizontal pass/stores.
  - Emission is software-pipelined: chunk i's horizontal stage is emitted
    after chunk i+1's vertical tree so carry/stash DMA latencies and the
    ACT-engine shifted copy hide under tree compute.
"""

import numpy as np

B, H, W = 16, 1024, 1024
NCORES = 8
IMGS = 2            # images per core
RB = 16             # rows per partition
CHUNKS = [344, 344, 336]
assert sum(CHUNKS) == W
NCH = len(CHUNKS)
UM = max(CHUNKS) + 8

_CACHE = {}


def _build(reps=1):
    import concourse.bacc as bacc
    import concourse.tile as tile
    import concourse.mybir as mybir

    bf16 = mybir.dt.bfloat16

    nc = bacc.Bacc("TRN2", target_bir_lowering=False, debug=False, num_devices=1)
    x = nc.dram_tensor("x", [IMGS, H, W], bf16, kind="ExternalInput").ap()
    y = nc.dram_tensor("y", [IMGS, H, W], bf16, kind="ExternalOutput").ap()

    xv = [x[i].rearrange("(q r) c -> q r c", r=RB) for i in range(IMGS)]
    chunk_off = np.cumsum([0] + CHUNKS[:-1]).tolist()

    with tile.TileContext(nc) as tc:
        with tc.tile_pool(name="pt", bufs=1) as pt:

            def t3(rows, tag):
                t = pt.tile([128, rows * UM], bf16, tag=tag)
                return t.rearrange("p (r u) -> p r u", u=UM)

            XT = [t3(17, "x0"), t3(17, "x1")]
            T2T = t3(18, "t2")
            T4T = t3(20, "t4")
            T8T = t3(17, "t8")
            R9T = [t3(16, "r90"), t3(16, "r91")]
            YT = [t3(16, "y0"), t3(16, "y1")]
            H2T = t3(16, "h2")
            H4T = t3(16, "h4")
            H8T = t3(16, "h8")
            OT = [t3(16, "o0"), t3(16, "o1")]
            TPT = [t3(4, "tp0"), t3(4, "tp1")]

            # carry rows start zero; partitions 0:126 are rewritten by the
            # per-chunk carry DMAs, the image-bottom partitions (126,127)
            # keep the zeros forever (partition-sliced memsets are not legal
            # BIR, so clear all 128 partitions)
            for t, r0, r1 in ((XT[0], 16, 17), (XT[1], 16, 17), (T2T, 16, 18),
                              (T4T, 16, 20), (T8T, 16, 17)):
                nc.gpsimd.memset(t[:, r0:r1, :], 0.0)

            def load(it):
                ch = it % NCH
                cw = CHUNKS[ch]
                u = cw + 8
                c0 = chunk_off[ch]
                clo = max(0, c0 - 4)
                chi = min(W, c0 + cw + 4)
                ncols = chi - clo
                ulo = clo - (c0 - 4)
                x3 = XT[it % 2]
                if ulo > 0:
                    nc.gpsimd.memset(x3[:, 0:RB, 0:ulo], 0.0)
                if ulo + ncols < u:
                    nc.gpsimd.memset(x3[:, 0:RB, ulo + ncols:u], 0.0)
                row_groups = [(0, 4), (4, 8), (8, 12), (12, RB)] if it == 0 else [(0, RB)]
                for rlo, rhi in row_groups:
                    for img in range(IMGS):
                        nc.scalar.dma_start(
                            out=x3[img:img + 127:2, rlo:rhi, ulo:ulo + ncols],
                            in_=xv[img][:, rlo:rhi, clo:chi],
                        )
                return x3

            def emit_tree(it, x3, last=False):
                s = it % 2
                cw = CHUNKS[it % NCH]
                u = cw + 8
                sync = nc.sync

                # X carry: row 16 <- next band's row 0 (both images at once)
                if it == 0:
                    # first chunk: T2 in row-quarters so compute starts while
                    # the very first load is still streaming in
                    nc.vector.tensor_max(T2T[:, 0:3, 0:u], x3[:, 0:3, 0:u], x3[:, 1:4, 0:u])
                    sync.dma_start(out=x3[0:126, 16:17, 0:u], in_=x3[2:128, 0:1, 0:u])
                    nc.vector.tensor_max(T2T[:, 3:7, 0:u], x3[:, 3:7, 0:u], x3[:, 4:8, 0:u])
                    nc.vector.tensor_max(T2T[:, 7:11, 0:u], x3[:, 7:11, 0:u], x3[:, 8:12, 0:u])
                    nc.vector.tensor_max(T2T[:, 11:16, 0:u], x3[:, 11:16, 0:u], x3[:, 12:17, 0:u])
                else:
                    sync.dma_start(out=x3[0:126, 16:17, 0:u], in_=x3[2:128, 0:1, 0:u])
                    nc.vector.tensor_max(T2T[:, 0:16, 0:u], x3[:, 0:16, 0:u], x3[:, 1:17, 0:u])

                sync.dma_start(out=T2T[0:126, 16:18, 0:u], in_=T2T[2:128, 0:2, 0:u])
                nc.vector.tensor_max(T4T[:, 0:16, 0:u], T2T[:, 0:16, 0:u], T2T[:, 2:18, 0:u])

                # top output rows 0..2 for both images (partitions 0,1 = q0)
                tp = TPT[s]
                nc.vector.tensor_max(tp[0:2, 0:1, 0:u], T4T[0:2, 0:1, 0:u], x3[0:2, 4:5, 0:u])
                nc.vector.tensor_max(tp[0:2, 1:2, 0:u], T4T[0:2, 0:1, 0:u], T2T[0:2, 4:5, 0:u])
                nc.vector.tensor_max(tp[0:2, 2:3, 0:u], T4T[0:2, 0:1, 0:u], T4T[0:2, 3:4, 0:u])

                sync.dma_start(out=T4T[0:126, 16:20, 0:u], in_=T4T[2:128, 0:4, 0:u])
                nc.vector.tensor_max(T8T[:, 0:16, 0:u], T4T[:, 0:16, 0:u], T4T[:, 4:20, 0:u])

                nc.scalar.copy(tp[0:2, 3:4, 0:u], T8T[0:2, 0:1, 0:u])

                sync.dma_start(out=T8T[0:126, 16:17, 0:u], in_=T8T[2:128, 0:1, 0:u])
                r9 = R9T[s]
                if last:
                    # split R9 + the ACT shifted copy by row halves so the
                    # final horizontal stage starts as early as possible
                    nc.vector.tensor_max(r9[:, 0:8, 0:u], T8T[:, 0:8, 0:u], T8T[:, 1:9, 0:u])
                    nc.scalar.copy(YT[s][:, 0:8, 0:cw + 6], r9[:, 0:8, 1:cw + 7])
                    nc.vector.tensor_max(r9[:, 8:16, 0:u], T8T[:, 8:16, 0:u], T8T[:, 9:17, 0:u])
                    sync.dma_start(out=r9[126:128, 12:16, 0:u], in_=tp[0:2, 0:4, 0:u])
                    nc.scalar.copy(YT[s][:, 8:16, 0:cw + 6], r9[:, 8:16, 1:cw + 7])
                else:
                    nc.vector.tensor_max(r9[:, 0:16, 0:u], T8T[:, 0:16, 0:u], T8T[:, 1:17, 0:u])
                    # odd horizontal shift on the ACT engine: Y[m] = R9[m+1].
                    # rows 0:12 don't overlap the stash, so they copy while the
                    # stash DMA is still in flight; only rows 12:16 wait on it.
                    nc.scalar.copy(YT[s][:, 0:12, 0:cw + 6], r9[:, 0:12, 1:cw + 7])
                    # stash top rows into the unused (p 126/127, r 12..15) slots
                    sync.dma_start(out=r9[126:128, 12:16, 0:u], in_=tp[0:2, 0:4, 0:u])
                    nc.scalar.copy(YT[s][:, 12:16, 0:cw + 6], r9[:, 12:16, 1:cw + 7])

            def emit_hstage(it, last=False):
                s = it % 2
                ch = it % NCH
                cw = CHUNKS[ch]
                c0 = chunk_off[ch]
                r9 = R9T[s]
                o3 = OT[s]
                ymains = [
                    y[img][4:4 + 63 * RB, c0:c0 + cw].rearrange("(q r) c -> q r c", r=RB)
                    for img in range(IMGS)
                ]

                def store_main(rlo, rhi, split=False):
                    for img in range(IMGS):
                        # on the drain path, route one store via SWDGE (Pool)
                        # so the two final stores don't serialize on HWDGE
                        eng = nc.gpsimd if (split and img == 1) else nc.sync
                        eng.dma_start(
                            out=ymains[img][:, rlo:rhi, :],
                            in_=o3[img:img + 125:2, rlo:rhi, 0:cw],
                        )

                def store_tail():
                    # bottom rows 1012..1023 at (p 126/127, r 0..11); SWDGE
                    # (Pool) path keeps these small stores off the HWDGE queue
                    nc.gpsimd.dma_start(
                        out=y[:, 1012:1024, c0:c0 + cw], in_=o3[126:128, 0:12, 0:cw]
                    )

                def store_top():
                    # top rows 0..3 from the stash (p 126/127, r 12..15)
                    nc.gpsimd.dma_start(
                        out=y[:, 0:4, c0:c0 + cw], in_=o3[126:128, 12:16, 0:cw]
                    )

                def htree(hlo, hhi):
                    nc.vector.tensor_max(
                        H2T[:, hlo:hhi, 0:cw + 6],
                        r9[:, hlo:hhi, 0:cw + 6],
                        YT[s][:, hlo:hhi, 0:cw + 6],
                    )
                    nc.vector.tensor_max(
                        H4T[:, hlo:hhi, 0:cw + 4],
                        H2T[:, hlo:hhi, 0:cw + 4],
                        H2T[:, hlo:hhi, 2:cw + 6],
                    )
                    nc.vector.tensor_max(
                        H8T[:, hlo:hhi, 0:cw],
                        H4T[:, hlo:hhi, 0:cw],
                        H4T[:, hlo:hhi, 4:cw + 4],
                    )

                def merge(hlo, hhi):
                    nc.vector.tensor_max(
                        o3[:, hlo:hhi, 0:cw],
                        H8T[:, hlo:hhi, 0:cw],
                        r9[:, hlo:hhi, 8:cw + 8],
                    )

                if not last:
                    htree(0, 16)
                    merge(0, 16)
                    store_main(0, 16)
                    store_tail()
                    store_top()
                else:
                    # final chunk: drain in halves/quarters so stores overlap
                    # the remaining merges instead of queueing after them
                    htree(0, 8)
                    merge(0, 8)
                    store_main(0, 8)
                    htree(8, 16)
                    merge(8, 12)
                    store_main(8, 12)
                    store_tail()
                    merge(12, 16)
                    store_main(12, 16, split=True)
                    store_top()

            niter = NCH * reps
            xp = {0: load(0)}
            for it in range(niter):
                if it + 1 < niter:
                    xp[it + 1] = load(it + 1)
                emit_tree(it, xp.pop(it), last=(it == niter - 1))
                if it > 0:
                    emit_hstage(it - 1)
            emit_hstage(niter - 1, last=True)

    nc.compile()
    return nc


def kernel(label):
    import ml_dtypes

    lab = np.ascontiguousarray(
        np.asarray(label, dtype=np.float32).reshape(B, H, W)
    ).astype(ml_dtypes.bfloat16)
    if "nc" not in _CACHE:
        _CACHE["nc"] = _build()
    nc = _CACHE["nc"]

    from concourse.bass_utils import run_bass_kernel_spmd

    in_maps = [{"x": lab[IMGS * c:IMGS * (c + 1)]} for c in range(NCORES)]
    res = run_bass_kernel_spmd(nc, in_maps, core_ids=list(range(NCORES)))
    out = np.concatenate(
        [np.asarray(res.results[c]["y"]).astype(np.float32) for c in range(NCORES)],
        axis=0,
    )
    return out.reshape(B, H, W, 1)


# revision 20
# speedup vs baseline: 1.1046x; 1.0683x over previous
"""9x9 morphological dilation (sliding-window max, SAME padding) on Trainium2.

Input : label (16, 1024, 1024, 1) float32, values in [0, 1).
Output: same shape; out[b,i,j] = max over the 9x9 window centered at (i,j),
        clipped to the image (cv2-style border handling for dilate).

Strategy (per NeuronCore; batch is data-parallel over 8 cores, 2 images/core):
  - All device compute and HBM traffic is bf16.  max() commutes with
    round-to-nearest, so out == rn_bf16(exact out): rel err <= 2^-9 ~ 0.2%,
    far inside the 2e-2 gate.  The host converts f32->bf16 on the way in and
    bf16->f32 on the way out; device DMA traffic is halved and every DVE
    tensor_tensor op runs in the 2x_1P perf mode (2 elem/cycle/lane), which
    fp32 tensor_tensor cannot use.
  - Layout: partition p holds img = p%2, row-band q = p//2 (image rows
    16q..16q+15); free dim is r-major with u = cw+12 padded columns per
    chunk (+-4 halo, zero pads at image edges; zero is a valid -inf
    substitute since inputs >= 0).  Interleaving the two images on even/odd
    partitions makes "next row-band" = "partition p+2" for BOTH images, so
    every cross-partition carry is ONE partition-shifted DMA (and the
    image-bottom partitions 126/127 keep permanently-zero carry rows).
  - Each tree tile carries extra rows at the end (X:+1, T2:+2, T4:+4,
    T8:+1) that hold the next band's leading rows, so every tree level is a
    single full-tile tensor_max (no separate boundary op).
  - Vertical 9-max: log tree (row shifts +1,+2,+4,+1 applied to X,T2,T4,T8).
    Row shifts keep the column offset 0, so all operands stay 4B-aligned ->
    2x mode.
  - Horizontal 9-max: log tree with column shifts +1,+2,+4,+8.  A +1 bf16
    shift is 2B-misaligned and would drop the whole op to 1x, so the odd
    shift is materialized once per chunk as a shifted copy on the (otherwise
    idle) Activation engine: Y[m] = R9[m+1].  The rest (H2=max(R9,Y),
    H4=max(H2,H2>>2), H8=max(H4,H4>>4), OUT=max(H8,R9>>8)) is all even ->
    2x on the DVE.
  - HWDGE descriptor generation is a single serialized device costing
    ~625ns per dma_start regardless of size, so DMAs are maximally merged:
    per chunk 2 loads, 4 carries, 1 top-row stash, 4 stores (2 of them on
    the Pool/SWDGE path, which bypasses HWDGE).
  - R9[r] covers rows R..R+8 -> output row R+4; recentering happens in the
    store offsets.  Output rows 0..3 (clipped top windows) are built from
    tree intermediates and stashed into the otherwise-unused rows
    (p in {126,127}, r>=12) so they ride the same horizontal pass/stores.
  - Emission is software-pipelined: chunk i's horizontal stage is emitted
    after chunk i+1's vertical tree so carry/stash DMA latencies and the
    ACT-engine shifted copy hide under tree compute.
"""

import numpy as np

B, H, W = 16, 1024, 1024
NCORES = 8
IMGS = 2            # images per core
RB = 16             # rows per partition
CHUNKS = [512, 512]
assert sum(CHUNKS) == W
NCH = len(CHUNKS)
UM = max(CHUNKS) + 12

_CACHE = {}


def _build(reps=1):
    import concourse.bacc as bacc
    import concourse.tile as tile
    import concourse.mybir as mybir

    bf16 = mybir.dt.bfloat16

    nc = bacc.Bacc("TRN2", target_bir_lowering=False, debug=False, num_devices=1)
    x = nc.dram_tensor("x", [IMGS, H, W], bf16, kind="ExternalInput").ap()
    y = nc.dram_tensor("y", [IMGS, H, W], bf16, kind="ExternalOutput").ap()

    xv = [x[i].rearrange("(q r) c -> q r c", r=RB) for i in range(IMGS)]
    chunk_off = np.cumsum([0] + CHUNKS[:-1]).tolist()

    with tile.TileContext(nc) as tc:
        with tc.tile_pool(name="pt", bufs=1) as pt:

            def t3(rows, tag):
                t = pt.tile([128, rows * UM], bf16, tag=tag)
                return t.rearrange("p (r u) -> p r u", u=UM)

            XT = [t3(17, "x0"), t3(17, "x1")]
            T2T = t3(18, "t2")
            T4T = t3(20, "t4")
            T8T = t3(17, "t8")
            R9T = [t3(16, "r90"), t3(16, "r91")]
            YT = [t3(16, "y0"), t3(16, "y1")]
            H2T = t3(16, "h2")
            H4T = t3(16, "h4")
            # SBUF is tight at cw=512: H8 overwrites H2T (H8 reads only H4),
            # the merged output overwrites H4T (reads only H8 + R9), and the
            # 4-row top-prefix tile is shared across the ping-pong
            H8T = H2T
            OT = [H4T, H4T]
            _tp = t3(4, "tp0")
            TPT = [_tp, _tp]

            # carry rows start zero; partitions 0:126 are rewritten by the
            # per-chunk carry DMAs, the image-bottom partitions (126,127)
            # keep the zeros forever (partition-sliced memsets are not legal
            # BIR, so clear all 128 partitions)
            for t, r0, r1 in ((XT[0], 16, 17), (XT[1], 16, 17), (T2T, 16, 18),
                              (T4T, 16, 20), (T8T, 16, 17)):
                nc.gpsimd.memset(t[:, r0:r1, :], 0.0)

            def load(it):
                ch = it % NCH
                cw = CHUNKS[ch]
                u = cw + 12
                c0 = chunk_off[ch]
                clo = max(0, c0 - 4)
                chi = min(W, c0 + cw + 8)
                ncols = chi - clo
                ulo = clo - (c0 - 4)
                x3 = XT[it % 2]
                if ulo > 0:
                    nc.gpsimd.memset(x3[:, 0:RB, 0:ulo], 0.0)
                if ulo + ncols < u:
                    nc.gpsimd.memset(x3[:, 0:RB, ulo + ncols:u], 0.0)
                row_groups = [(0, 4), (4, 8), (8, 12), (12, RB)] if it == 0 else [(0, RB)]
                for rlo, rhi in row_groups:
                    for img in range(IMGS):
                        nc.scalar.dma_start(
                            out=x3[img:img + 127:2, rlo:rhi, ulo:ulo + ncols],
                            in_=xv[img][:, rlo:rhi, clo:chi],
                        )
                return x3

            def emit_tree(it, x3, last=False):
                s = it % 2
                cw = CHUNKS[it % NCH]
                u = cw + 12
                sync = nc.sync

                # X carry: row 16 <- next band's row 0 (both images at once)
                if it == 0:
                    # first chunk: T2 in row-quarters so compute starts while
                    # the very first load is still streaming in
                    nc.vector.tensor_max(T2T[:, 0:3, 0:u], x3[:, 0:3, 0:u], x3[:, 1:4, 0:u])
                    sync.dma_start(out=x3[0:126, 16:17, 0:u], in_=x3[2:128, 0:1, 0:u])
                    nc.vector.tensor_max(T2T[:, 3:7, 0:u], x3[:, 3:7, 0:u], x3[:, 4:8, 0:u])
                    nc.vector.tensor_max(T2T[:, 7:11, 0:u], x3[:, 7:11, 0:u], x3[:, 8:12, 0:u])
                    nc.vector.tensor_max(T2T[:, 11:16, 0:u], x3[:, 11:16, 0:u], x3[:, 12:17, 0:u])
                else:
                    sync.dma_start(out=x3[0:126, 16:17, 0:u], in_=x3[2:128, 0:1, 0:u])
                    nc.vector.tensor_max(T2T[:, 0:16, 0:u], x3[:, 0:16, 0:u], x3[:, 1:17, 0:u])

                sync.dma_start(out=T2T[0:126, 16:18, 0:u], in_=T2T[2:128, 0:2, 0:u])
                nc.vector.tensor_max(T4T[:, 0:16, 0:u], T2T[:, 0:16, 0:u], T2T[:, 2:18, 0:u])

                # top output rows 0..2 for both images (partitions 0,1 = q0)
                tp = TPT[s]
                nc.vector.tensor_max(tp[0:2, 0:1, 0:u], T4T[0:2, 0:1, 0:u], x3[0:2, 4:5, 0:u])
                nc.vector.tensor_max(tp[0:2, 1:2, 0:u], T4T[0:2, 0:1, 0:u], T2T[0:2, 4:5, 0:u])
                nc.vector.tensor_max(tp[0:2, 2:3, 0:u], T4T[0:2, 0:1, 0:u], T4T[0:2, 3:4, 0:u])

                sync.dma_start(out=T4T[0:126, 16:20, 0:u], in_=T4T[2:128, 0:4, 0:u])
                nc.vector.tensor_max(T8T[:, 0:16, 0:u], T4T[:, 0:16, 0:u], T4T[:, 4:20, 0:u])

                nc.scalar.copy(tp[0:2, 3:4, 0:u], T8T[0:2, 0:1, 0:u])

                sync.dma_start(out=T8T[0:126, 16:17, 0:u], in_=T8T[2:128, 0:1, 0:u])
                r9 = R9T[s]
                if last:
                    # split R9 + the ACT shifted copy by row halves so the
                    # final horizontal stage starts as early as possible
                    nc.vector.tensor_max(r9[:, 0:8, 0:u], T8T[:, 0:8, 0:u], T8T[:, 1:9, 0:u])
                    nc.scalar.copy(YT[s][:, 0:8, 0:cw + 6], r9[:, 0:8, 1:cw + 7])
                    nc.vector.tensor_max(r9[:, 8:16, 0:u], T8T[:, 8:16, 0:u], T8T[:, 9:17, 0:u])
                    sync.dma_start(out=r9[126:128, 12:16, 0:u], in_=tp[0:2, 0:4, 0:u])
                    nc.scalar.copy(YT[s][:, 8:16, 0:cw + 6], r9[:, 8:16, 1:cw + 7])
                else:
                    nc.vector.tensor_max(r9[:, 0:16, 0:u], T8T[:, 0:16, 0:u], T8T[:, 1:17, 0:u])
                    # odd horizontal shift on the ACT engine: Y[m] = R9[m+1].
                    # rows 0:12 don't overlap the stash, so they copy while the
                    # stash DMA is still in flight; only rows 12:16 wait on it.
                    nc.scalar.copy(YT[s][:, 0:12, 0:cw + 6], r9[:, 0:12, 1:cw + 7])
                    # stash top rows into the unused (p 126/127, r 12..15) slots
                    sync.dma_start(out=r9[126:128, 12:16, 0:u], in_=tp[0:2, 0:4, 0:u])
                    nc.scalar.copy(YT[s][:, 12:16, 0:cw + 6], r9[:, 12:16, 1:cw + 7])

            def emit_hstage(it, last=False):
                s = it % 2
                ch = it % NCH
                cw = CHUNKS[ch]
                c0 = chunk_off[ch]
                r9 = R9T[s]
                o3 = OT[s]
                ymains = [
                    y[img][4:4 + 63 * RB, c0:c0 + cw].rearrange("(q r) c -> q r c", r=RB)
                    for img in range(IMGS)
                ]

                def store_main(rlo, rhi, split=False):
                    for img in range(IMGS):
                        # on the drain path, route one store via SWDGE (Pool)
                        # so the two final stores don't serialize on HWDGE
                        eng = nc.gpsimd if (split and img == 1) else nc.sync
                        eng.dma_start(
                            out=ymains[img][:, rlo:rhi, :],
                            in_=o3[img:img + 125:2, rlo:rhi, 0:cw],
                        )

                def store_tail():
                    # bottom rows 1012..1023 at (p 126/127, r 0..11); SWDGE
                    # (Pool) path keeps these small stores off the HWDGE queue
                    nc.gpsimd.dma_start(
                        out=y[:, 1012:1024, c0:c0 + cw], in_=o3[126:128, 0:12, 0:cw]
                    )

                def store_top():
                    # top rows 0..3 from the stash (p 126/127, r 12..15)
                    nc.gpsimd.dma_start(
                        out=y[:, 0:4, c0:c0 + cw], in_=o3[126:128, 12:16, 0:cw]
                    )

                def htree(hlo, hhi):
                    nc.vector.tensor_max(
                        H2T[:, hlo:hhi, 0:cw + 6],
                        r9[:, hlo:hhi, 0:cw + 6],
                        YT[s][:, hlo:hhi, 0:cw + 6],
                    )
                    nc.vector.tensor_max(
                        H4T[:, hlo:hhi, 0:cw + 4],
                        H2T[:, hlo:hhi, 0:cw + 4],
                        H2T[:, hlo:hhi, 2:cw + 6],
                    )
                    nc.vector.tensor_max(
                        H8T[:, hlo:hhi, 0:cw],
                        H4T[:, hlo:hhi, 0:cw],
                        H4T[:, hlo:hhi, 4:cw + 4],
                    )

                def merge(hlo, hhi):
                    nc.vector.tensor_max(
                        o3[:, hlo:hhi, 0:cw],
                        H8T[:, hlo:hhi, 0:cw],
                        r9[:, hlo:hhi, 8:cw + 8],
                    )

                if not last:
                    htree(0, 16)
                    merge(0, 16)
                    store_main(0, 16)
                    store_tail()
                    store_top()
                else:
                    # final chunk: drain in halves/quarters so stores overlap
                    # the remaining merges instead of queueing after them
                    htree(0, 8)
                    merge(0, 8)
                    store_main(0, 8)
                    htree(8, 16)
                    merge(8, 12)
                    store_main(8, 12)
                    store_tail()
                    merge(12, 16)
                    store_main(12, 16, split=True)
                    store_top()

            niter = NCH * reps
            xp = {0: load(0)}
            for it in range(niter):
                if it + 1 < niter:
                    xp[it + 1] = load(it + 1)
                emit_tree(it, xp.pop(it), last=(it == niter - 1))
                if it > 0:
                    emit_hstage(it - 1)
            emit_hstage(niter - 1, last=True)

    nc.compile()
    return nc


def kernel(label):
    import ml_dtypes

    lab = np.ascontiguousarray(
        np.asarray(label, dtype=np.float32).reshape(B, H, W)
    ).astype(ml_dtypes.bfloat16)
    if "nc" not in _CACHE:
        _CACHE["nc"] = _build()
    nc = _CACHE["nc"]

    from concourse.bass_utils import run_bass_kernel_spmd

    in_maps = [{"x": lab[IMGS * c:IMGS * (c + 1)]} for c in range(NCORES)]
    res = run_bass_kernel_spmd(nc, in_maps, core_ids=list(range(NCORES)))
    out = np.concatenate(
        [np.asarray(res.results[c]["y"]).astype(np.float32) for c in range(NCORES)],
        axis=0,
    )
    return out.reshape(B, H, W, 1)
